# revision 1
# baseline (speedup 1.0000x reference)
# Trainium2 Bass kernel for nn_Decoder_14568529068506 (gnn_message_passing).
#
# Reference computation (per scene s of 32, P=48 peds):
#   rel[i,j]  = obs[j] - obs[i]                  (P,P,2T)   2T=16
#   emb       = rel @ W_se.T                     (P,P,512)
#   emb      *= tile(traj_weight[s])             (P,P,512)
#   x         = concat([emb, h[j]], -1)          (P,P,576)
#   x1        = relu(x @ W1.T + b1)              (P,P,512)
#   x2        = relu(x1 @ W2.T + b2)             (P,P,1024)
#   out[s,i]  = max_j x2[i,j]                    (P,1024)
#
# Kernel restructuring (validated exactly in fp32 numpy):
#  * The tiled traj_weight multiply + spatial embedding + W1 are fused:
#      out1[d,row] = sum_{(ct,g)} Wf[d,(ct,g)] * tw[row,ct] * rel[row,g]
#    with Wf[d, ct*16+g] = sum_{k%2==c} W1[d, t*64+k] * W_se[t*64+k, g].
#    So MLP1 contracts over 256 "rel2" features (+64 h features) instead
#    of 576, and the (P,P,512) embedding is never materialized.
#  * rel2 = tw_rep * rel_rep is built feature-major on 128 partitions:
#      rel_rep = obs_rep.T @ D   (D = +-1 pairwise difference matrix)
#      tw_rep  = R.T @ twT       (R = 0/1 replication matrix)
#    i.e. three cheap matmuls + two vector multiplies per row block.
#  * The h-state part of MLP1 rides as a third K=64 accumulation matmul
#    whose rhs (h broadcast over i) is built once per scene.
#  * relu/bias commute with max-pool, so MLP2 outputs are max-pooled
#    straight out of PSUM; bias+relu are applied post-pool on [128,48].
#  * Matmuls run in bf16 (1 cycle/row; separate LDWEIGHTS path). PSUM
#    accumulation stays fp32; only matmul operands are rounded.
#
# Sharding: scenes are data-parallel across the 8 cores (4 scenes each);
# weights replicated; the (192,1024) per-core outputs are concatenated on
# the host (no collectives needed).

import numpy as np

S, P, T, E, H = 32, 48, 8, 64, 64
D1, D2 = 512, 1024
B = S * P
NCORES = 8
SC = S // NCORES          # scenes per core
NB = 6                    # row blocks per scene
NBLK = P * P // NB        # 384 columns (pairs) per block = 8 i-groups x 48 j
IB = NBLK // P            # i-groups per block (8)


def _host_constants(W_se, W1, W2, b1, b2):
    """Precompute fused weights + structural constant matrices (fp32)."""
    W_se = np.asarray(W_se, np.float32)
    W1 = np.asarray(W1, np.float32)
    W2 = np.asarray(W2, np.float32)
    b1 = np.asarray(b1, np.float32)
    b2 = np.asarray(b2, np.float32)

    W1e, W1h = W1[:, :512], W1[:, 512:]
    Wf = np.zeros((D1, 256), np.float32)
    for c in range(2):
        for t in range(T):
            ct = c * 8 + t
            f = t * 64 + np.arange(c, 64, 2)
            Wf[:, ct * 16:(ct + 1) * 16] = W1e[:, f] @ W_se[f, :]

    Dm = np.zeros((P, P * P), np.float32)
    ii, jj = np.meshgrid(np.arange(P), np.arange(P), indexing="ij")
    rows = (ii * P + jj).ravel()
    np.add.at(Dm, (jj.ravel(), rows), 1.0)
    np.add.at(Dm, (ii.ravel(), rows), -1.0)

    R0 = np.zeros((16, 128), np.float32)
    R1 = np.zeros((16, 128), np.float32)
    for ct in range(8):
        R0[ct, ct * 16:(ct + 1) * 16] = 1.0
        R1[ct + 8, ct * 16:(ct + 1) * 16] = 1.0

    # lhsT tile layouts: [128, kTiles, M] so DMAs are contiguous
    Wf_sb = np.ascontiguousarray(Wf.T.reshape(2, 128, D1).transpose(1, 0, 2))
    W1h_sb = np.ascontiguousarray(W1h.T)                     # (64, 512)
    W2_sb = np.ascontiguousarray(W2.T.reshape(4, 128, D2).transpose(1, 0, 2))
    b1_sb = np.ascontiguousarray(b1.reshape(4, 128).T)       # (128, 4)
    b2_sb = np.ascontiguousarray(b2.reshape(8, 128).T)       # (128, 8)
    ident = np.eye(128, dtype=np.float32)
    return dict(Wf_sb=Wf_sb, W1h_sb=W1h_sb, W2_sb=W2_sb, b1_sb=b1_sb,
                b2_sb=b2_sb, Dm=Dm, R0=R0, R1=R1, ident=ident)


def build_program(n_scenes=SC):
    """Emit the per-core Bass/Tile program. Returns the compiled Bacc.

    Built on bacc.Bacc (not raw bass.Bass): Bacc.compile() runs the
    TRN2 sync legalization (move_matmul_waits_to_ldweights +
    generate_event_semaphores) that splits multi-semaphore waits —
    hardware allows at most one sync-wait per instruction.
    """
    from contextlib import ExitStack
    import concourse.bacc as bacc
    import concourse.tile as tile
    from concourse import mybir
    from concourse.alu_op_type import AluOpType

    f32 = mybir.dt.float32
    bf16 = mybir.dt.bfloat16
    AF = mybir.ActivationFunctionType
    AX = mybir.AxisListType

    nc = bacc.Bacc("TRN2", target_bir_lowering=False, debug=False)

    # ---- DRAM parameters -------------------------------------------------
    d_obs = nc.dram_tensor("obs_rm", [n_scenes * P, 16], bf16, kind="ExternalInput")
    d_tw = nc.dram_tensor("twT", [n_scenes, 16, P * P], bf16, kind="ExternalInput")
    d_h = nc.dram_tensor("h_fm", [n_scenes, 64, P], bf16, kind="ExternalInput")
    d_Dm = nc.dram_tensor("Dm", [P, P * P], bf16, kind="ExternalInput")
    d_R0 = nc.dram_tensor("R0", [16, 128], bf16, kind="ExternalInput")
    d_R1 = nc.dram_tensor("R1", [16, 128], bf16, kind="ExternalInput")
    d_Wf = nc.dram_tensor("Wf_sb", [128, 2, D1], bf16, kind="ExternalInput")
    d_W1h = nc.dram_tensor("W1h_sb", [64, D1], bf16, kind="ExternalInput")
    d_W2 = nc.dram_tensor("W2_sb", [128, 4, D2], bf16, kind="ExternalInput")
    d_b1 = nc.dram_tensor("b1_sb", [128, 4], f32, kind="ExternalInput")
    d_b2 = nc.dram_tensor("b2_sb", [128, 8], f32, kind="ExternalInput")
    d_id = nc.dram_tensor("ident", [128, 128], f32, kind="ExternalInput")
    d_out = nc.dram_tensor("out", [n_scenes * P, D2], f32, kind="ExternalOutput")

    with ExitStack() as ctx:
        tc = ctx.enter_context(tile.TileContext(nc))
        consts = ctx.enter_context(tc.tile_pool(name="consts", bufs=1))
        tw_pool = ctx.enter_context(tc.tile_pool(name="tw", bufs=2))
        scene_pool = ctx.enter_context(tc.tile_pool(name="scene", bufs=2))
        blk_pool = ctx.enter_context(tc.tile_pool(name="blk", bufs=3))
        pp = ctx.enter_context(tc.tile_pool(name="pp", bufs=2, space="PSUM"))
        p1 = ctx.enter_context(tc.tile_pool(name="p1", bufs=2, space="PSUM"))
        p2 = ctx.enter_context(tc.tile_pool(name="p2", bufs=4, space="PSUM"))

        # ---- small resident constants (big weights stream in after the
        # first scene's data so the first matmuls start sooner) ----------
        Dm_sb = consts.tile([P, P * P], bf16)
        nc.sync.dma_start(Dm_sb[:], d_Dm[:])
        Rb_sb = consts.tile([128, 128], bf16)
        nc.sync.dma_start(Rb_sb[64:80, :], d_R0[:])
        nc.sync.dma_start(Rb_sb[96:112, :], d_R1[:])
        b1_sb = consts.tile([128, 4], f32)
        nc.sync.dma_start(b1_sb[:], d_b1[:])
        b2_sb = consts.tile([128, 8], f32)
        nc.sync.dma_start(b2_sb[:], d_b2[:])
        id_sb = consts.tile([128, 128], f32)
        nc.sync.dma_start(id_sb[:], d_id[:])
        zero_sb = consts.tile([128, P], f32)
        nc.vector.memset(zero_sb[:], 0.0)
        Wf_sb = consts.tile([128, 2, D1], bf16)
        W1h_sb = consts.tile([64, D1], bf16)
        W2_sb = consts.tile([128, 4, D2], bf16)

        def load_weights():
            nc.sync.dma_start(Wf_sb[:, 0], d_Wf[:, 0])
            nc.sync.dma_start(Wf_sb[:, 1], d_Wf[:, 1])
            nc.sync.dma_start(W1h_sb[:], d_W1h[:])
            for k in range(4):
                nc.sync.dma_start(W2_sb[:, k], d_W2[:, k])

        blocks = [(s, b) for s in range(n_scenes) for b in range(NB)]
        state = {}   # per-scene tiles
        mlp_q = []   # software pipeline: deferred MLP stage

        def scene_setup(s):
            tw = tw_pool.tile([128, P * P], bf16, tag="tw")
            nc.sync.dma_start(tw[64:80, :], d_tw[s])
            nc.sync.dma_start(tw[96:112, :], d_tw[s])
            # replicate on-chip (a broadcast DMA would emit one packet per
            # repeat per partition - hundreds of tiny descriptors)
            obs_c = scene_pool.tile([P, 16], bf16, tag="obs_c")
            nc.sync.dma_start(obs_c[:], d_obs[s * P:(s + 1) * P, :])
            obs_rep = scene_pool.tile([P, 128], bf16, tag="obs_rep")
            nc.vector.tensor_copy(
                obs_rep[:].rearrange("p (r g) -> p r g", r=8),
                obs_c[:].unsqueeze(1).broadcast_to([P, 8, 16]))
            # h broadcast over i: hj_fm[:, ii*P + j] = h_fm[s, :, j]
            h_c = scene_pool.tile([64, P], bf16, tag="h_c")
            nc.sync.dma_start(h_c[:], d_h[s])
            hj_fm = scene_pool.tile([64, NBLK], bf16, tag="hj_fm")
            nc.vector.tensor_copy(
                hj_fm[:].rearrange("p (r j) -> p r j", r=IB),
                h_c[:].unsqueeze(1).broadcast_to([64, IB, P]))
            pooled = scene_pool.tile([128, 4, 2 * P], f32, tag="pooled")
            state[s] = dict(tw=tw, obs_rep=obs_rep, hj_fm=hj_fm, pooled=pooled)

        def prep(s, b):
            st = state[s]
            c0 = b * NBLK
            rel_ps = pp.tile([128, NBLK], f32, tag="pp")
            nc.tensor.matmul(rel_ps[:], st["obs_rep"][:],
                             Dm_sb[:, c0:c0 + NBLK], start=True, stop=True,
                             tile_position=(0, 0))
            tw0_ps = pp.tile([128, NBLK], f32, tag="pp")
            nc.tensor.matmul(tw0_ps[:], Rb_sb[64:80, :],
                             st["tw"][64:80, c0:c0 + NBLK], start=True,
                             stop=True, tile_position=(64, 0))
            tw1_ps = pp.tile([128, NBLK], f32, tag="pp")
            nc.tensor.matmul(tw1_ps[:], Rb_sb[96:112, :],
                             st["tw"][96:112, c0:c0 + NBLK], start=True,
                             stop=True, tile_position=(96, 0))
            rel_sb = blk_pool.tile([128, NBLK], f32, tag="rel_sb")
            nc.vector.tensor_copy(rel_sb[:], rel_ps[:])
            rel2_0 = blk_pool.tile([128, NBLK], bf16, tag="rel2_0")
            nc.vector.tensor_tensor(rel2_0[:], tw0_ps[:], rel_sb[:], AluOpType.mult)
            rel2_1 = blk_pool.tile([128, NBLK], bf16, tag="rel2_1")
            nc.vector.tensor_tensor(rel2_1[:], tw1_ps[:], rel_sb[:], AluOpType.mult)
            return dict(rel2_0=rel2_0, rel2_1=rel2_1, s=s, b=b)

        def mlp1(job):
            s, b = job["s"], job["b"]
            st = state[s]
            r20 = job["rel2_0"][:]
            r21 = job["rel2_1"][:]
            x1 = blk_pool.tile([128, 4, NBLK], bf16, tag="x1")
            for m in range(4):
                p1t = p1.tile([128, NBLK], f32, tag="p1")
                nc.tensor.matmul(p1t[:], Wf_sb[:, 0, m * 128:(m + 1) * 128],
                                 r20, start=True, stop=False)
                nc.tensor.matmul(p1t[:], Wf_sb[:, 1, m * 128:(m + 1) * 128],
                                 r21, start=False, stop=False)
                nc.tensor.matmul(p1t[:], W1h_sb[:, m * 128:(m + 1) * 128],
                                 st["hj_fm"][:], start=False, stop=True)
                nc.scalar.activation(x1[:, m, :], p1t[:], AF.Relu,
                                     bias=b1_sb[:, m:m + 1])
            job["x1"] = x1

        def mlp2(job):
            s, b = job["s"], job["b"]
            st = state[s]
            x1 = job["x1"]
            last = b == NB - 1
            for mm in range(8):
                p2t = p2.tile([128, NBLK], f32, tag="p2")
                for k in range(4):
                    nc.tensor.matmul(
                        p2t[:], W2_sb[:, k, mm * 128:(mm + 1) * 128],
                        x1[:, k, :], start=(k == 0), stop=(k == 3))
                dst = st["pooled"][:, mm // 2,
                                   (mm % 2) * P + b * IB:(mm % 2) * P + (b + 1) * IB]
                nc.vector.tensor_reduce(
                    dst, p2t[:].rearrange("p (i j) -> p i j", i=IB),
                    axis=AX.X, op=AluOpType.max)
                # one-group delay so PE doesn't stall on the pair's
                # reduce -> transpose chain (no delay on the final scene,
                # where it would only stretch the kernel tail)
                if last and mm % 2 == 1:
                    if s == n_scenes - 1:
                        finish_pair(s, st, mm // 2)
                    elif mm >= 3:
                        finish_pair(s, st, (mm - 3) // 2)
            if last:
                if s != n_scenes - 1:
                    finish_pair(s, st, 2)
                    finish_pair(s, st, 3)
                state.pop(s)

        def finish_pair(s, st, pi):
            """Scene output for m-tile pair pi: bias+relu post-pool,
            transpose to row-major, stage to SBUF, DMA out."""
            pooled = st["pooled"]
            for half in range(2):
                mm = 2 * pi + half
                sl = pooled[:, pi, half * P:(half + 1) * P]
                nc.vector.scalar_tensor_tensor(
                    sl, sl, b2_sb[:, mm:mm + 1], zero_sb[:],
                    op0=AluOpType.add, op1=AluOpType.max)
            tps = p1.tile([128, NBLK], f32, tag="p1")
            nc.tensor.transpose(tps[:2 * P, :128], pooled[:, pi, :], id_sb[:])
            ot = scene_pool.tile([2 * P, 128], f32, tag="ot")
            nc.vector.tensor_copy(ot[:], tps[:2 * P, :128])
            nc.sync.dma_start(
                d_out[s * P:(s + 1) * P, (2 * pi) * 128:(2 * pi + 1) * 128],
                ot[:P, :])
            nc.sync.dma_start(
                d_out[s * P:(s + 1) * P, (2 * pi + 1) * 128:(2 * pi + 2) * 128],
                ot[P:2 * P, :])

        # two-deep software pipeline on PE:
        #   ... prep(i)  mlp1(i-1)  mlp2(i-2) ...
        # so x1 is ready a full block before MLP2 consumes it and PSUM
        # slot recycling has a block of slack; scene data is prefetched
        # one block before the scene starts
        scene_setup(0)
        for idx, (s, b) in enumerate(blocks):
            if b == NB - 2 and s + 1 < n_scenes:
                scene_setup(s + 1)
            if idx == 0:
                load_weights()
            mlp_q.append(prep(s, b))
            if len(mlp_q) > 1:
                mlp1(mlp_q[-2])
            if len(mlp_q) > 2:
                mlp2(mlp_q.pop(0))
        mlp1(mlp_q[-1])
        mlp2(mlp_q.pop(0))
        mlp2(mlp_q.pop(0))

    nc.compile()
    return nc


def _host_inputs(h_states, traj, traj_weight, consts, n_scenes=SC):
    """Slice + lay out per-core input maps (matmul operands cast to bf16)."""
    import ml_dtypes
    bf = ml_dtypes.bfloat16
    h_states = np.asarray(h_states, np.float32)
    traj = np.asarray(traj, np.float32)
    traj_weight = np.asarray(traj_weight, np.float32)

    obs_full = np.ascontiguousarray(
        traj[:T].transpose(1, 0, 2).reshape(B, 2 * T))          # (B,16) g=t*2+c
    h_full = h_states.reshape(S, P, H)

    consts = dict(consts)
    for k in ("Wf_sb", "W1h_sb", "W2_sb", "Dm", "R0", "R1"):
        consts[k] = consts[k].astype(bf)

    in_maps = []
    for core in range(NCORES):
        s0 = core * n_scenes
        sl = slice(s0, s0 + n_scenes)
        twT = np.ascontiguousarray(
            traj_weight[sl].transpose(0, 2, 3, 1).reshape(n_scenes, 16, P * P)
        ).astype(bf)
        h_fm = np.ascontiguousarray(h_full[sl].transpose(0, 2, 1)).astype(bf)
        obs_rm = np.ascontiguousarray(
            obs_full[s0 * P:(s0 + n_scenes) * P]).astype(bf)
        m = dict(obs_rm=obs_rm, twT=twT, h_fm=h_fm)
        m.update(consts)
        in_maps.append(m)
    return in_maps


def kernel(h_states, seq_start_end, end_pos, traj, traj_weight,
           mlp_pre_pool_dim_0, W_se, b_se, W1, b1, W2, b2):
    import sys
    if '/opt/trn_rl_repo' not in sys.path:
        sys.path.insert(0, '/opt/trn_rl_repo')
    from concourse.bass_utils import run_bass_kernel_spmd

    consts = _host_constants(W_se, W1, W2, b1, b2)
    in_maps = _host_inputs(h_states, traj, traj_weight, consts)
    nc = build_program(SC)
    res = run_bass_kernel_spmd(nc, in_maps, list(range(NCORES)))
    out = np.concatenate([res.results[i]["out"] for i in range(NCORES)], axis=0)
    return out.astype(np.float32)



# revision 4
# speedup vs baseline: 1.3261x; 1.3261x over previous
# Trainium2 Bass kernel for nn_Decoder_14568529068506 (gnn_message_passing).
#
# Reference computation (per scene s of 32, P=48 peds):
#   rel[i,j]  = obs[j] - obs[i]                  (P,P,2T)   2T=16
#   emb       = rel @ W_se.T                     (P,P,512)
#   emb      *= tile(traj_weight[s])             (P,P,512)
#   x         = concat([emb, h[j]], -1)          (P,P,576)
#   x1        = relu(x @ W1.T + b1)              (P,P,512)
#   x2        = relu(x1 @ W2.T + b2)             (P,P,1024)
#   out[s,i]  = max_j x2[i,j]                    (P,1024)
#
# Kernel restructuring (validated in fp32 numpy):
#  * traj_weight tiling + spatial embedding + W1 fused on the host:
#      out1[d,row] = sum_{(ct,g)} Wf[d,(ct,g)] * tw[row,ct] * rel[row,g]
#    so MLP1 contracts over 256 "rel2" features; the (P,P,512) embedding
#    never exists.
#  * The h-state part of MLP1 is column-constant within a scene:
#    y_h = W1h @ h + b1 is computed once per scene as a tiny N=48 matmul
#    (bias via a constant-1 input row) and injected per block by one DVE
#    add, so the per-pair MLP1 is two K=128 matmuls per m-tile - the PE
#    stream has a single weight row-group config (no switch bubbles).
#  * All data replication (tw -> 128 partitions, obs -> (r,g)x(i,j)
#    layouts) is pure layout, done on the HOST and shipped as ONE packed
#    DMA per scene; the arithmetic rel = obsJ - obsI, rel2 = tw * rel
#    runs on GPSIMD (SBUF-only engine).
#  * relu/bias commute with max-pool; MLP2 outputs are max-pooled out of
#    PSUM by DVE (one reduce per 128-row m-tile); bias+relu run
#    post-pool on the Act engine into an f16 [128,8,48] tile, one output
#    DMA per scene.
#  * fp16 operands everywhere (1 cycle/row on PE, same speed as bf16,
#    8x the mantissa accuracy of bf16). PSUM accumulation stays fp32.
#  * The host does the final output transpose to (B, 1024) f32 -
#    layout only.
#
# Sharding: scenes are data-parallel across the 8 cores (4 scenes each);
# weights replicated; per-core outputs are concatenated on the host.

import numpy as np

S, P, T, E, H = 32, 48, 8, 64, 64
D1, D2 = 512, 1024
B = S * P
NCORES = 8
SC = S // NCORES          # scenes per core
NB = 6                    # row blocks per scene
NBLK = P * P // NB        # 384 columns (pairs) per block = 8 i-groups x 48 j
IB = NBLK // P            # i-groups per block (8)
PP = P * P                # 2304
# packed per-scene input layout: [obsI | obsJ | tw0 | tw1], each PP cols
O_I, O_J, O_T0, O_T1 = 0, PP, 2 * PP, 3 * PP


def _host_constants(W_se, W1, W2, b1, b2):
    """Fused weights + lhsT layouts (fp32; cast to f16 in _host_inputs)."""
    W_se = np.asarray(W_se, np.float32)
    W1 = np.asarray(W1, np.float32)
    W2 = np.asarray(W2, np.float32)
    b1 = np.asarray(b1, np.float32)
    b2 = np.asarray(b2, np.float32)

    W1e, W1h = W1[:, :512], W1[:, 512:]
    Wf = np.zeros((D1, 256), np.float32)
    for c in range(2):
        for t in range(T):
            ct = c * 8 + t
            f = t * 64 + np.arange(c, 64, 2)
            Wf[:, ct * 16:(ct + 1) * 16] = W1e[:, f] @ W_se[f, :]

    # lhsT tile layouts: [K(128 part), kTiles, M]
    Wf_sb = np.ascontiguousarray(Wf.T.reshape(2, 128, D1).transpose(1, 0, 2))
    # W1h padded to K=128: row 64 is the constant-1 bias row carrying b1
    W1hp = np.zeros((128, D1), np.float32)
    W1hp[:64] = W1h.T
    W1hp[64] = b1
    W2_sb = np.ascontiguousarray(W2.T.reshape(4, 128, D2).transpose(1, 0, 2))
    b2_sb = np.ascontiguousarray(b2.reshape(8, 128).T)       # (128, 8)
    return dict(Wf_sb=Wf_sb, W1hp=W1hp, W2_sb=W2_sb, b2_sb=b2_sb)


def build_program(n_scenes=SC):
    """Emit the per-core Bass/Tile program. Returns the compiled Bacc."""
    from contextlib import ExitStack
    import concourse.bacc as bacc
    import concourse.tile as tile
    from concourse import mybir
    from concourse.alu_op_type import AluOpType

    f32 = mybir.dt.float32
    f16 = mybir.dt.float16
    AF = mybir.ActivationFunctionType
    AX = mybir.AxisListType

    nc = bacc.Bacc("TRN2", target_bir_lowering=False, debug=False)

    # ---- DRAM parameters -------------------------------------------------
    d_combo = nc.dram_tensor("combo", [n_scenes, 128, 4 * PP], f16, kind="ExternalInput")
    d_h = nc.dram_tensor("h_scp", [n_scenes, 128, P], f16, kind="ExternalInput")
    d_Wf = nc.dram_tensor("Wf_sb", [128, 2, D1], f16, kind="ExternalInput")
    d_W1hp = nc.dram_tensor("W1hp", [128, D1], f16, kind="ExternalInput")
    d_W2 = nc.dram_tensor("W2_sb", [128, 4, D2], f16, kind="ExternalInput")
    d_b2 = nc.dram_tensor("b2_sb", [128, 8], f32, kind="ExternalInput")
    d_out = nc.dram_tensor("out", [n_scenes, 128, 8, P], f16, kind="ExternalOutput")

    with ExitStack() as ctx:
        tc = ctx.enter_context(tile.TileContext(nc))
        consts = ctx.enter_context(tc.tile_pool(name="consts", bufs=1))
        scene_pool = ctx.enter_context(tc.tile_pool(name="scene", bufs=2))
        blk_pool = ctx.enter_context(tc.tile_pool(name="blk", bufs=3))
        p1 = ctx.enter_context(tc.tile_pool(name="p1", bufs=3, space="PSUM"))
        p2 = ctx.enter_context(tc.tile_pool(name="p2", bufs=4, space="PSUM"))
        pyh = ctx.enter_context(tc.tile_pool(name="pyh", bufs=1, space="PSUM"))

        # weight DMAs ordered for the startup critical path: W1hp (y_h of
        # scene 0) and Wf (first MLP1) first; W2 needed ~2us later.
        W1hp_sb = consts.tile([128, D1], f16)
        nc.sync.dma_start(W1hp_sb[:], d_W1hp[:])
        Wf_sb = consts.tile([128, 2, D1], f16)
        nc.sync.dma_start(Wf_sb[:], d_Wf[:])
        W2_sb = consts.tile([128, 4, D2], f16)
        b2_sb = consts.tile([128, 8], f32)

        blocks = [(s, b) for s in range(n_scenes) for b in range(NB)]
        state = {}   # per-scene tiles
        mlp_q = []   # software pipeline: deferred MLP2 stage

        def setup_dma(s):
            """DMA the scene's host-packed tiles; GPSIMD builds rel2.
            Scene 0 arrives in quarters with the prep on DVE so the first
            block is ready as early as possible."""
            h_scp = scene_pool.tile([128, P], f16, tag="h_scp")
            nc.sync.dma_start(h_scp[:], d_h[s])
            combo = scene_pool.tile([128, 4 * PP], f16, tag="combo")
            rel = scene_pool.tile([128, PP], f16, tag="rel")
            rel2 = scene_pool.tile([128, 2, PP], f16, tag="rel2")

            if s == 0:
                eng, nch = nc.vector, 4
                cv = combo[:].rearrange("p (t c) -> p t c", t=4)
                dv = d_combo[s].rearrange("p (t c) -> p t c", t=4)
                for q in range(nch):
                    cs = slice(q * (PP // nch), (q + 1) * (PP // nch))
                    nc.sync.dma_start(cv[:, :, cs], dv[:, :, cs])
            else:
                eng, nch = nc.gpsimd, 2
                nc.sync.dma_start(combo[:], d_combo[s])
            for ch in range(nch):
                c0, w = ch * (PP // nch), PP // nch
                eng.tensor_tensor(rel[:, c0:c0 + w],
                                  combo[:, O_J + c0:O_J + c0 + w],
                                  combo[:, O_I + c0:O_I + c0 + w],
                                  AluOpType.subtract)
                eng.tensor_tensor(rel2[:, 0, c0:c0 + w], rel[:, c0:c0 + w],
                                  combo[:, O_T0 + c0:O_T0 + c0 + w],
                                  AluOpType.mult)
                eng.tensor_tensor(rel2[:, 1, c0:c0 + w], rel[:, c0:c0 + w],
                                  combo[:, O_T1 + c0:O_T1 + c0 + w],
                                  AluOpType.mult)

            pooled = scene_pool.tile([128, 8, P], f32, tag="pooled")
            out_sb = scene_pool.tile([128, 8, P], f16, tag="out_sb")
            state[s] = dict(rel2=rel2, h_scp=h_scp, pooled=pooled,
                            out_sb=out_sb)

        def setup_yh(s):
            """y_h[m] = W1h @ h + b1 as four tiny N=48 matmuls."""
            st = state[s]
            yh_ps = pyh.tile([128, 4, P], f32, tag="yh")
            for m in range(4):
                nc.tensor.matmul(yh_ps[:, m, :],
                                 W1hp_sb[:, m * 128:(m + 1) * 128],
                                 st["h_scp"][:], start=True, stop=True)
            yh_sb = scene_pool.tile([128, 4, P], f32, tag="yh_sb")
            nc.vector.tensor_copy(yh_sb[:], yh_ps[:])
            st["yh_sb"] = yh_sb

        def mlp1(s, b):
            st = state[s]
            c0 = b * NBLK
            x1 = blk_pool.tile([128, 4, NBLK], f16, tag="x1")
            for m in range(4):
                p1t = p1.tile([128, NBLK], f32, tag="p1")
                nc.tensor.matmul(p1t[:], Wf_sb[:, 0, m * 128:(m + 1) * 128],
                                 st["rel2"][:, 0, c0:c0 + NBLK],
                                 start=True, stop=False)
                nc.tensor.matmul(p1t[:], Wf_sb[:, 1, m * 128:(m + 1) * 128],
                                 st["rel2"][:, 1, c0:c0 + NBLK],
                                 start=False, stop=True)
                x1p = blk_pool.tile([128, NBLK], f32, tag="x1p")
                nc.vector.tensor_tensor(
                    x1p[:].rearrange("p (i j) -> p i j", i=IB),
                    p1t[:].rearrange("p (i j) -> p i j", i=IB),
                    st["yh_sb"][:, m, :].unsqueeze(1).broadcast_to([128, IB, P]),
                    AluOpType.add)
                nc.scalar.activation(x1[:, m, :], x1p[:], AF.Relu)
            return x1

        def mlp2(s, b, x1):
            st = state[s]
            for mm in range(8):
                p2t = p2.tile([128, 512], f32, tag="p2")
                for k in range(4):
                    nc.tensor.matmul(
                        p2t[:, :NBLK],
                        W2_sb[:, k, mm * 128:(mm + 1) * 128],
                        x1[:, k, :], start=(k == 0), stop=(k == 3))
                nc.vector.tensor_reduce(
                    st["pooled"][:, mm, b * IB:(b + 1) * IB],
                    p2t[:, :NBLK].rearrange("p (i j) -> p i j", i=IB),
                    axis=AX.X, op=AluOpType.max)
            if b == NB - 1:
                for mm in range(8):
                    nc.scalar.activation(
                        st["out_sb"][:, mm, :], st["pooled"][:, mm, :],
                        AF.Relu, bias=b2_sb[:, mm:mm + 1])
                nc.sync.dma_start(d_out[s], st["out_sb"][:])
                state.pop(s)

        # two-deep software pipeline on PE: ... mlp1(i)  mlp2(i-1) ...
        # scene DMAs+prep prefetched a full scene early (GPSIMD prep of a
        # scene takes ~34us); y_h matmuls land mid-scene so the PE never
        # waits on their input DMAs.
        setup_dma(0)
        setup_yh(0)
        nc.sync.dma_start(W2_sb[:], d_W2[:])
        nc.sync.dma_start(b2_sb[:], d_b2[:])
        for idx, (s, b) in enumerate(blocks):
            if b == 0 and s + 1 < n_scenes:
                setup_dma(s + 1)
            if b == 3 and s + 1 < n_scenes:
                setup_yh(s + 1)
            mlp_q.append((s, b, mlp1(s, b)))
            if len(mlp_q) > 1:
                mlp2(*mlp_q.pop(0))
        mlp2(*mlp_q.pop(0))

    nc.compile()
    return nc


def _host_inputs(h_states, traj, traj_weight, consts, n_scenes=SC):
    """Slice + lay out per-core input maps (all matmul operands f16)."""
    f16 = np.float16
    h_states = np.asarray(h_states, np.float32)
    traj = np.asarray(traj, np.float32)
    traj_weight = np.asarray(traj_weight, np.float32)

    obs = traj[:T].transpose(1, 0, 2).reshape(S, P, 2 * T)   # (S,P,16) g=t*2+c
    h_full = h_states.reshape(S, P, H)

    # obsT[s, p=(r*16+g), j] = obs[s, j, g]   (replica r = 0..7)
    obsT = np.tile(obs.transpose(0, 2, 1), (1, 8, 1))        # (S,128,48)
    combo = np.empty((S, 128, 4 * PP), f16)
    combo[:, :, O_J:O_J + PP] = np.tile(obsT, (1, 1, P))     # [..., i*P+j]=obs[j]
    combo[:, :, O_I:O_I + PP] = np.repeat(obsT, P, axis=2)   # [..., i*P+j]=obs[i]
    # twX[s, p=(r*16+g), col] = tw[s, ct, col], ct = r (tw0) / 8+r (tw1)
    twT = np.ascontiguousarray(
        traj_weight.transpose(0, 2, 3, 1).reshape(S, 16, PP))
    combo[:, :, O_T0:O_T0 + PP] = np.repeat(twT[:, 0:8], 16, axis=1)
    combo[:, :, O_T1:O_T1 + PP] = np.repeat(twT[:, 8:16], 16, axis=1)
    # h_scp[s, k, j] = h[s, j, k] padded to K=128 with the bias row at 64
    h_scp = np.zeros((S, 128, P), f16)
    h_scp[:, :64] = h_full.transpose(0, 2, 1)
    h_scp[:, 64] = 1.0

    c16 = {k: (v.astype(f16) if k in ("Wf_sb", "W1hp", "W2_sb") else v)
           for k, v in consts.items()}

    in_maps = []
    for core in range(NCORES):
        sl = slice(core * n_scenes, (core + 1) * n_scenes)
        m = dict(combo=np.ascontiguousarray(combo[sl]),
                 h_scp=np.ascontiguousarray(h_scp[sl]))
        m.update(c16)
        in_maps.append(m)
    return in_maps


def kernel(h_states, seq_start_end, end_pos, traj, traj_weight,
           mlp_pre_pool_dim_0, W_se, b_se, W1, b1, W2, b2):
    import sys
    if '/opt/trn_rl_repo' not in sys.path:
        sys.path.insert(0, '/opt/trn_rl_repo')
    from concourse.bass_utils import run_bass_kernel_spmd

    consts = _host_constants(W_se, W1, W2, b1, b2)
    in_maps = _host_inputs(h_states, traj, traj_weight, consts)
    nc = build_program(SC)
    res = run_bass_kernel_spmd(nc, in_maps, list(range(NCORES)))
    # device output: [n_scenes, 128 (d%128), 8 (d//128), 48 (i)] f16 per core
    parts = []
    for i in range(NCORES):
        o = np.asarray(res.results[i]["out"], np.float32)
        parts.append(o.transpose(0, 3, 2, 1).reshape(SC * P, D2))
    return np.concatenate(parts, axis=0)


# revision 8
# speedup vs baseline: 1.3589x; 1.0248x over previous
# Trainium2 Bass kernel for nn_Decoder_14568529068506 (gnn_message_passing).
#
# Reference computation (per scene s of 32, P=48 peds):
#   rel[i,j]  = obs[j] - obs[i]                  (P,P,2T)   2T=16
#   emb       = rel @ W_se.T                     (P,P,512)
#   emb      *= tile(traj_weight[s])             (P,P,512)
#   x         = concat([emb, h[j]], -1)          (P,P,576)
#   x1        = relu(x @ W1.T + b1)              (P,P,512)
#   x2        = relu(x1 @ W2.T + b2)             (P,P,1024)
#   out[s,i]  = max_j x2[i,j]                    (P,1024)
#
# Kernel restructuring (validated in fp32 numpy):
#  * traj_weight tiling + spatial embedding + W1 fused on the host:
#      out1[d,row] = sum_{(ct,g)} Wf[d,(ct,g)] * tw[row,ct] * rel[row,g]
#    so MLP1 contracts over 256 "rel2" features; the (P,P,512) embedding
#    never exists.
#  * The h-state part of MLP1 is column-constant within a scene:
#    y_h = W1h @ h + b1 is computed once per scene as a tiny N=48 matmul
#    (bias via a constant-1 input row) and injected per block by one DVE
#    add, so the per-pair MLP1 is two K=128 matmuls per m-tile - the PE
#    stream has a single weight row-group config (no switch bubbles).
#  * All data replication (tw -> 128 partitions, obs -> (r,g)x(i,j)
#    layouts) is pure layout, done on the HOST and shipped as ONE packed
#    DMA per scene; the arithmetic rel = obsJ - obsI, rel2 = tw * rel
#    runs on GPSIMD (SBUF-only engine).
#  * relu/bias commute with max-pool; MLP2 outputs are max-pooled out of
#    PSUM by DVE (one reduce per 128-row m-tile); bias+relu run
#    post-pool on the Act engine into an f16 [128,8,48] tile, one output
#    DMA per scene.
#  * fp16 operands everywhere (1 cycle/row on PE, same speed as bf16,
#    8x the mantissa accuracy of bf16). PSUM accumulation stays fp32.
#  * The host does the final output transpose to (B, 1024) f32 -
#    layout only.
#
# Sharding: scenes are data-parallel across the 8 cores (4 scenes each);
# weights replicated; per-core outputs are concatenated on the host.

import numpy as np

S, P, T, E, H = 32, 48, 8, 64, 64
D1, D2 = 512, 1024
B = S * P
NCORES = 8
SC = S // NCORES          # scenes per core
NB = 6                    # row blocks per scene
NBLK = P * P // NB        # 384 columns (pairs) per block = 8 i-groups x 48 j
IB = NBLK // P            # i-groups per block (8)
PP = P * P                # 2304
# packed per-scene input layout: [obsI | obsJ | tw0 | tw1], each PP cols
O_I, O_J, O_T0, O_T1 = 0, PP, 2 * PP, 3 * PP


def _host_constants(W_se, W1, W2, b1, b2):
    """Fused weights + lhsT layouts (fp32; cast to f16 in _host_inputs)."""
    W_se = np.asarray(W_se, np.float32)
    W1 = np.asarray(W1, np.float32)
    W2 = np.asarray(W2, np.float32)
    b1 = np.asarray(b1, np.float32)
    b2 = np.asarray(b2, np.float32)

    W1e, W1h = W1[:, :512], W1[:, 512:]
    Wf = np.zeros((D1, 256), np.float32)
    for c in range(2):
        for t in range(T):
            ct = c * 8 + t
            f = t * 64 + np.arange(c, 64, 2)
            Wf[:, ct * 16:(ct + 1) * 16] = W1e[:, f] @ W_se[f, :]

    # lhsT tile layouts: [K(128 part), kTiles, M]
    Wf_sb = np.ascontiguousarray(Wf.T.reshape(2, 128, D1).transpose(1, 0, 2))
    # W1h padded to K=128: row 64 is the constant-1 bias row carrying b1
    W1hp = np.zeros((128, D1), np.float32)
    W1hp[:64] = W1h.T
    W1hp[64] = b1
    W2_sb = np.ascontiguousarray(W2.T.reshape(4, 128, D2).transpose(1, 0, 2))
    b2_sb = np.ascontiguousarray(b2.reshape(8, 128).T)       # (128, 8)
    return dict(Wf_sb=Wf_sb, W1hp=W1hp, W2_sb=W2_sb, b2_sb=b2_sb)


def build_program(n_scenes=SC):
    """Emit the per-core Bass/Tile program. Returns the compiled Bacc."""
    from contextlib import ExitStack
    import concourse.bacc as bacc
    import concourse.tile as tile
    from concourse import mybir
    from concourse.alu_op_type import AluOpType

    f32 = mybir.dt.float32
    f16 = mybir.dt.float16
    AF = mybir.ActivationFunctionType
    AX = mybir.AxisListType

    nc = bacc.Bacc("TRN2", target_bir_lowering=False, debug=False)

    # ---- DRAM parameters -------------------------------------------------
    d_combo = nc.dram_tensor("combo", [n_scenes, 128, 4 * PP], f16, kind="ExternalInput")
    d_h = nc.dram_tensor("h_scp", [n_scenes, 128, P], f16, kind="ExternalInput")
    d_Wf = nc.dram_tensor("Wf_sb", [128, 2, D1], f16, kind="ExternalInput")
    d_W1hp = nc.dram_tensor("W1hp", [128, D1], f16, kind="ExternalInput")
    d_W2 = nc.dram_tensor("W2_sb", [128, 4, D2], f16, kind="ExternalInput")
    d_b2 = nc.dram_tensor("b2_sb", [128, 8], f32, kind="ExternalInput")
    d_out = nc.dram_tensor("out", [n_scenes, 128, 8, P], f16, kind="ExternalOutput")

    with ExitStack() as ctx:
        tc = ctx.enter_context(tile.TileContext(nc))
        consts = ctx.enter_context(tc.tile_pool(name="consts", bufs=1))
        scene_pool = ctx.enter_context(tc.tile_pool(name="scene", bufs=2))
        blk_pool = ctx.enter_context(tc.tile_pool(name="blk", bufs=3))
        p1 = ctx.enter_context(tc.tile_pool(name="p1", bufs=3, space="PSUM"))
        p2 = ctx.enter_context(tc.tile_pool(name="p2", bufs=4, space="PSUM"))
        pyh = ctx.enter_context(tc.tile_pool(name="pyh", bufs=1, space="PSUM"))

        # weight DMAs ordered for the startup critical path: W1hp (y_h of
        # scene 0) first; Wf (first MLP1) and W2 are interleaved with the
        # scene-0 chunk DMAs below, each just ahead of first use.
        W1hp_sb = consts.tile([128, D1], f16)
        nc.sync.dma_start(W1hp_sb[:], d_W1hp[:])
        Wf_sb = consts.tile([128, 2, D1], f16)
        W2_sb = consts.tile([128, 4, D2], f16)
        b2_sb = consts.tile([128, 8], f32)

        blocks = [(s, b) for s in range(n_scenes) for b in range(NB)]
        state = {}   # per-scene tiles
        mlp_q = []   # software pipeline: deferred MLP2 stage

        def prep_chunk(eng, st, c0, w):
            combo, rel, rel2 = st["combo"], st["rel"], st["rel2"]
            eng.tensor_tensor(rel[:, c0:c0 + w],
                              combo[:, O_J + c0:O_J + c0 + w],
                              combo[:, O_I + c0:O_I + c0 + w],
                              AluOpType.subtract)
            eng.tensor_tensor(rel2[:, 0, c0:c0 + w], rel[:, c0:c0 + w],
                              combo[:, O_T0 + c0:O_T0 + c0 + w],
                              AluOpType.mult)
            eng.tensor_tensor(rel2[:, 1, c0:c0 + w], rel[:, c0:c0 + w],
                              combo[:, O_T1 + c0:O_T1 + c0 + w],
                              AluOpType.mult)

        def new_scene(s):
            st = state[s] = dict(
                h_scp=scene_pool.tile([128, P], f16, tag="h_scp", name="h_scp"),
                combo=scene_pool.tile([128, 4 * PP], f16, tag="combo", name="combo"),
                rel=scene_pool.tile([128, PP], f16, tag="rel", name="rel"),
                rel2=scene_pool.tile([128, 2, PP], f16, tag="rel2", name="rel2"),
                pooled=scene_pool.tile([128, 8, P], f32, tag="pooled", name="pooled"),
                out_sb=scene_pool.tile([128, 8, P], f16, tag="out_sb", name="out_sb"))
            nc.sync.dma_start(st["h_scp"][:], d_h[s])
            return st

        def setup_dma(s):
            """DMA the scene's host-packed tiles; GPSIMD builds rel2."""
            st = new_scene(s)
            nc.sync.dma_start(st["combo"][:], d_combo[s])
            for ch in range(2):
                prep_chunk(nc.gpsimd, st, ch * (PP // 2), PP // 2)

        def setup_scene0():
            """Scene 0 arrives in block-aligned sixths; the first chunk's
            prep runs on DVE (fast, idle at start), the rest on GPSIMD.
            Wf/W2 weight DMAs are slotted right where the startup
            critical path wants them."""
            st = new_scene(0)
            cv = st["combo"][:].rearrange("p (t c) -> p t c", t=4)
            dv = d_combo[0].rearrange("p (t c) -> p t c", t=4)
            for ch in range(NB):
                cs = slice(ch * NBLK, (ch + 1) * NBLK)
                nc.sync.dma_start(cv[:, :, cs], dv[:, :, cs])
                prep_chunk(nc.vector if ch == 0 else nc.gpsimd,
                           st, ch * NBLK, NBLK)
                if ch == 0:
                    nc.sync.dma_start(Wf_sb[:], d_Wf[:])
                elif ch == 1:
                    nc.sync.dma_start(W2_sb[:], d_W2[:])

        def setup_yh(s):
            """y_h[m] = W1h @ h + b1 as four tiny N=48 matmuls."""
            st = state[s]
            yh_ps = pyh.tile([128, 4, P], f32, tag="yh")
            for m in range(4):
                nc.tensor.matmul(yh_ps[:, m, :],
                                 W1hp_sb[:, m * 128:(m + 1) * 128],
                                 st["h_scp"][:], start=True, stop=True)
            yh_sb = scene_pool.tile([128, 4, P], f32, tag="yh_sb")
            nc.vector.tensor_copy(yh_sb[:], yh_ps[:])
            st["yh_sb"] = yh_sb

        def mlp1(s, b):
            st = state[s]
            c0 = b * NBLK
            x1 = blk_pool.tile([128, 4, NBLK], f16, tag="x1")
            for m in range(4):
                p1t = p1.tile([128, NBLK], f32, tag="p1")
                nc.tensor.matmul(p1t[:], Wf_sb[:, 0, m * 128:(m + 1) * 128],
                                 st["rel2"][:, 0, c0:c0 + NBLK],
                                 start=True, stop=False)
                nc.tensor.matmul(p1t[:], Wf_sb[:, 1, m * 128:(m + 1) * 128],
                                 st["rel2"][:, 1, c0:c0 + NBLK],
                                 start=False, stop=True)
                nc.vector.tensor_tensor(
                    x1[:, m, :].rearrange("p (i j) -> p i j", i=IB),
                    p1t[:].rearrange("p (i j) -> p i j", i=IB),
                    st["yh_sb"][:, m, :].unsqueeze(1).broadcast_to([128, IB, P]),
                    AluOpType.add)
                nc.scalar.activation(x1[:, m, :], x1[:, m, :], AF.Relu)
            return x1

        def mlp2(s, b, x1):
            st = state[s]
            for mm in range(8):
                p2t = p2.tile([128, 512], f32, tag="p2")
                for k in range(4):
                    nc.tensor.matmul(
                        p2t[:, :NBLK],
                        W2_sb[:, k, mm * 128:(mm + 1) * 128],
                        x1[:, k, :], start=(k == 0), stop=(k == 3))
                nc.vector.tensor_reduce(
                    st["pooled"][:, mm, b * IB:(b + 1) * IB],
                    p2t[:, :NBLK].rearrange("p (i j) -> p i j", i=IB),
                    axis=AX.X, op=AluOpType.max)
            if b == NB - 1:
                for mm in range(8):
                    nc.scalar.activation(
                        st["out_sb"][:, mm, :], st["pooled"][:, mm, :],
                        AF.Relu, bias=b2_sb[:, mm:mm + 1])
                nc.sync.dma_start(d_out[s], st["out_sb"][:])
                state.pop(s)

        # two-deep software pipeline on PE: ... mlp1(i)  mlp2(i-1) ...
        # scene DMAs+prep prefetched a full scene early (GPSIMD prep of a
        # scene takes ~34us); y_h matmuls land mid-scene so the PE never
        # waits on their input DMAs.
        setup_scene0()
        setup_yh(0)
        nc.sync.dma_start(b2_sb[:], d_b2[:])
        for idx, (s, b) in enumerate(blocks):
            if b == 0 and s + 1 < n_scenes:
                setup_dma(s + 1)
            if b == 3 and s + 1 < n_scenes:
                setup_yh(s + 1)
            mlp_q.append((s, b, mlp1(s, b)))
            if len(mlp_q) > 1:
                mlp2(*mlp_q.pop(0))
        mlp2(*mlp_q.pop(0))

    nc.compile()
    return nc


def _host_inputs(h_states, traj, traj_weight, consts, n_scenes=SC):
    """Slice + lay out per-core input maps (all matmul operands f16)."""
    f16 = np.float16
    h_states = np.asarray(h_states, np.float32)
    traj = np.asarray(traj, np.float32)
    traj_weight = np.asarray(traj_weight, np.float32)

    obs = traj[:T].transpose(1, 0, 2).reshape(S, P, 2 * T)   # (S,P,16) g=t*2+c
    h_full = h_states.reshape(S, P, H)

    # obsT[s, p=(r*16+g), j] = obs[s, j, g]   (replica r = 0..7)
    obsT = np.tile(obs.transpose(0, 2, 1), (1, 8, 1))        # (S,128,48)
    combo = np.empty((S, 128, 4 * PP), f16)
    combo[:, :, O_J:O_J + PP] = np.tile(obsT, (1, 1, P))     # [..., i*P+j]=obs[j]
    combo[:, :, O_I:O_I + PP] = np.repeat(obsT, P, axis=2)   # [..., i*P+j]=obs[i]
    # twX[s, p=(r*16+g), col] = tw[s, ct, col], ct = r (tw0) / 8+r (tw1)
    twT = np.ascontiguousarray(
        traj_weight.transpose(0, 2, 3, 1).reshape(S, 16, PP))
    combo[:, :, O_T0:O_T0 + PP] = np.repeat(twT[:, 0:8], 16, axis=1)
    combo[:, :, O_T1:O_T1 + PP] = np.repeat(twT[:, 8:16], 16, axis=1)
    # h_scp[s, k, j] = h[s, j, k] padded to K=128 with the bias row at 64
    h_scp = np.zeros((S, 128, P), f16)
    h_scp[:, :64] = h_full.transpose(0, 2, 1)
    h_scp[:, 64] = 1.0

    c16 = {k: (v.astype(f16) if k in ("Wf_sb", "W1hp", "W2_sb") else v)
           for k, v in consts.items()}

    in_maps = []
    for core in range(NCORES):
        sl = slice(core * n_scenes, (core + 1) * n_scenes)
        m = dict(combo=np.ascontiguousarray(combo[sl]),
                 h_scp=np.ascontiguousarray(h_scp[sl]))
        m.update(c16)
        in_maps.append(m)
    return in_maps


def kernel(h_states, seq_start_end, end_pos, traj, traj_weight,
           mlp_pre_pool_dim_0, W_se, b_se, W1, b1, W2, b2):
    import sys
    if '/opt/trn_rl_repo' not in sys.path:
        sys.path.insert(0, '/opt/trn_rl_repo')
    from concourse.bass_utils import run_bass_kernel_spmd

    consts = _host_constants(W_se, W1, W2, b1, b2)
    in_maps = _host_inputs(h_states, traj, traj_weight, consts)
    nc = build_program(SC)
    res = run_bass_kernel_spmd(nc, in_maps, list(range(NCORES)))
    # device output: [n_scenes, 128 (d%128), 8 (d//128), 48 (i)] f16 per core
    parts = []
    for i in range(NCORES):
        o = np.asarray(res.results[i]["out"], np.float32)
        parts.append(o.transpose(0, 3, 2, 1).reshape(SC * P, D2))
    return np.concatenate(parts, axis=0)


# revision 10
# speedup vs baseline: 1.3688x; 1.0073x over previous
# Trainium2 Bass kernel for nn_Decoder_14568529068506 (gnn_message_passing).
#
# Reference computation (per scene s of 32, P=48 peds):
#   rel[i,j]  = obs[j] - obs[i]                  (P,P,2T)   2T=16
#   emb       = rel @ W_se.T                     (P,P,512)
#   emb      *= tile(traj_weight[s])             (P,P,512)
#   x         = concat([emb, h[j]], -1)          (P,P,576)
#   x1        = relu(x @ W1.T + b1)              (P,P,512)
#   x2        = relu(x1 @ W2.T + b2)             (P,P,1024)
#   out[s,i]  = max_j x2[i,j]                    (P,1024)
#
# Kernel restructuring (validated in fp32 numpy):
#  * traj_weight tiling + spatial embedding + W1 fused on the host:
#      out1[d,row] = sum_{(ct,g)} Wf[d,(ct,g)] * tw[row,ct] * rel[row,g]
#    so MLP1 contracts over 256 "rel2" features; the (P,P,512) embedding
#    never exists.
#  * The h-state part of MLP1 is column-constant within a scene:
#    y_h = W1h @ h + b1 is computed once per scene as a tiny N=48 matmul
#    (bias via a constant-1 input row) and injected per block by one DVE
#    add, so the per-pair MLP1 is two K=128 matmuls per m-tile - the PE
#    stream has a single weight row-group config (no switch bubbles).
#  * All data replication (tw -> 128 partitions, obs -> (r,g)x(i,j)
#    layouts) is pure layout, done on the HOST and shipped as ONE packed
#    DMA per scene; the arithmetic rel = obsJ - obsI, rel2 = tw * rel
#    runs on GPSIMD (SBUF-only engine).
#  * relu/bias commute with max-pool; MLP2 outputs are max-pooled out of
#    PSUM by DVE (one reduce per 128-row m-tile); bias+relu run
#    post-pool on the Act engine into an f16 [128,8,48] tile, one output
#    DMA per scene.
#  * fp16 operands everywhere (1 cycle/row on PE, same speed as bf16,
#    8x the mantissa accuracy of bf16). PSUM accumulation stays fp32.
#  * The host does the final output transpose to (B, 1024) f32 -
#    layout only.
#
# Sharding: scenes are data-parallel across the 8 cores (4 scenes each);
# weights replicated; per-core outputs are concatenated on the host.

import numpy as np

S, P, T, E, H = 32, 48, 8, 64, 64
D1, D2 = 512, 1024
B = S * P
NCORES = 8
SC = S // NCORES          # scenes per core
NB = 6                    # row blocks per scene
NBLK = P * P // NB        # 384 columns (pairs) per block = 8 i-groups x 48 j
IB = NBLK // P            # i-groups per block (8)
PP = P * P                # 2304
# packed per-scene input layout: [obsI | obsJ | tw0 | tw1], each PP cols
O_I, O_J, O_T0, O_T1 = 0, PP, 2 * PP, 3 * PP


def _host_constants(W_se, W1, W2, b1, b2):
    """Fused weights + lhsT layouts (fp32; cast to f16 in _host_inputs)."""
    W_se = np.asarray(W_se, np.float32)
    W1 = np.asarray(W1, np.float32)
    W2 = np.asarray(W2, np.float32)
    b1 = np.asarray(b1, np.float32)
    b2 = np.asarray(b2, np.float32)

    W1e, W1h = W1[:, :512], W1[:, 512:]
    Wf = np.zeros((D1, 256), np.float32)
    for c in range(2):
        for t in range(T):
            ct = c * 8 + t
            f = t * 64 + np.arange(c, 64, 2)
            Wf[:, ct * 16:(ct + 1) * 16] = W1e[:, f] @ W_se[f, :]

    # lhsT tile layouts: [K(128 part), kTiles, M]
    Wf_sb = np.ascontiguousarray(Wf.T.reshape(2, 128, D1).transpose(1, 0, 2))
    # W1h padded to K=128: row 64 is the constant-1 bias row carrying b1
    W1hp = np.zeros((128, D1), np.float32)
    W1hp[:64] = W1h.T
    W1hp[64] = b1
    W2_sb = np.ascontiguousarray(W2.T.reshape(4, 128, D2).transpose(1, 0, 2))
    b2_sb = np.ascontiguousarray(b2.reshape(8, 128).T)       # (128, 8)
    return dict(Wf_sb=Wf_sb, W1hp=W1hp, W2_sb=W2_sb, b2_sb=b2_sb)


def build_program(n_scenes=SC):
    """Emit the per-core Bass/Tile program. Returns the compiled Bacc."""
    from contextlib import ExitStack
    import concourse.bacc as bacc
    import concourse.tile as tile
    from concourse import mybir
    from concourse.alu_op_type import AluOpType

    f32 = mybir.dt.float32
    f16 = mybir.dt.float16
    AF = mybir.ActivationFunctionType
    AX = mybir.AxisListType

    nc = bacc.Bacc("TRN2", target_bir_lowering=False, debug=False)

    # ---- DRAM parameters -------------------------------------------------
    d_combo = nc.dram_tensor("combo", [n_scenes, 128, 4 * PP], f16, kind="ExternalInput")
    d_h = nc.dram_tensor("h_scp", [n_scenes, 128, P], f16, kind="ExternalInput")
    d_Wf = nc.dram_tensor("Wf_sb", [128, 2, D1], f16, kind="ExternalInput")
    d_W1hp = nc.dram_tensor("W1hp", [128, D1], f16, kind="ExternalInput")
    d_W2 = nc.dram_tensor("W2_sb", [128, 4, D2], f16, kind="ExternalInput")
    d_b2 = nc.dram_tensor("b2_sb", [128, 8], f32, kind="ExternalInput")
    d_out = nc.dram_tensor("out", [n_scenes, 128, 8, P], f16, kind="ExternalOutput")

    with ExitStack() as ctx:
        tc = ctx.enter_context(tile.TileContext(nc))
        consts = ctx.enter_context(tc.tile_pool(name="consts", bufs=1))
        scene_pool = ctx.enter_context(tc.tile_pool(name="scene", bufs=2))
        blk_pool = ctx.enter_context(tc.tile_pool(name="blk", bufs=3))
        p1 = ctx.enter_context(tc.tile_pool(name="p1", bufs=3, space="PSUM"))
        p2 = ctx.enter_context(tc.tile_pool(name="p2", bufs=4, space="PSUM"))
        pyh = ctx.enter_context(tc.tile_pool(name="pyh", bufs=1, space="PSUM"))

        # weight DMAs ordered for the startup critical path: W1hp (y_h of
        # scene 0) first; Wf (first MLP1) and W2 are interleaved with the
        # scene-0 chunk DMAs below, each just ahead of first use.
        W1hp_sb = consts.tile([128, D1], f16)
        nc.sync.dma_start(W1hp_sb[:], d_W1hp[:])
        Wf_sb = consts.tile([128, 2, D1], f16)
        W2_sb = consts.tile([128, 4, D2], f16)
        b2_sb = consts.tile([128, 8], f32)

        blocks = [(s, b) for s in range(n_scenes) for b in range(NB)]
        state = {}   # per-scene tiles
        mlp_q = []   # software pipeline: deferred MLP2 stage

        def prep_chunk(eng, st, c0, w):
            combo, rel, rel2 = st["combo"], st["rel"], st["rel2"]
            eng.tensor_tensor(rel[:, c0:c0 + w],
                              combo[:, O_J + c0:O_J + c0 + w],
                              combo[:, O_I + c0:O_I + c0 + w],
                              AluOpType.subtract)
            eng.tensor_tensor(rel2[:, 0, c0:c0 + w], rel[:, c0:c0 + w],
                              combo[:, O_T0 + c0:O_T0 + c0 + w],
                              AluOpType.mult)
            eng.tensor_tensor(rel2[:, 1, c0:c0 + w], rel[:, c0:c0 + w],
                              combo[:, O_T1 + c0:O_T1 + c0 + w],
                              AluOpType.mult)

        def new_scene(s):
            st = state[s] = dict(
                h_scp=scene_pool.tile([128, P], f16, tag="h_scp", name="h_scp"),
                combo=scene_pool.tile([128, 4 * PP], f16, tag="combo", name="combo"),
                rel=scene_pool.tile([128, PP], f16, tag="rel", name="rel"),
                rel2=scene_pool.tile([128, 2, PP], f16, tag="rel2", name="rel2"),
                pooled=scene_pool.tile([128, 8, P], f32, tag="pooled", name="pooled"),
                out_sb=scene_pool.tile([128, 8, P], f16, tag="out_sb", name="out_sb"))
            nc.sync.dma_start(st["h_scp"][:], d_h[s])
            return st

        def setup_dma(s):
            """DMA the scene's host-packed tiles; GPSIMD builds rel2."""
            st = new_scene(s)
            nc.sync.dma_start(st["combo"][:], d_combo[s])
            for ch in range(2):
                prep_chunk(nc.gpsimd, st, ch * (PP // 2), PP // 2)

        def setup_scene0():
            """Scene 0 arrives in block-aligned sixths; the first chunk's
            prep runs on DVE (fast, idle at start), the rest on GPSIMD.
            Wf/W2 weight DMAs are slotted right where the startup
            critical path wants them."""
            st = new_scene(0)
            cv = st["combo"][:].rearrange("p (t c) -> p t c", t=4)
            dv = d_combo[0].rearrange("p (t c) -> p t c", t=4)
            for ch in range(NB):
                cs = slice(ch * NBLK, (ch + 1) * NBLK)
                nc.sync.dma_start(cv[:, :, cs], dv[:, :, cs])
                prep_chunk(nc.vector if ch <= 1 else nc.gpsimd,
                           st, ch * NBLK, NBLK)
                if ch == 0:
                    nc.sync.dma_start(Wf_sb[:], d_Wf[:])
                elif ch == 1:
                    nc.sync.dma_start(W2_sb[:], d_W2[:])

        def setup_yh(s):
            """y_h[m] = W1h @ h + b1 as four tiny N=48 matmuls."""
            st = state[s]
            yh_ps = pyh.tile([128, 4, P], f32, tag="yh")
            for m in range(4):
                nc.tensor.matmul(yh_ps[:, m, :],
                                 W1hp_sb[:, m * 128:(m + 1) * 128],
                                 st["h_scp"][:], start=True, stop=True)
            yh_sb = scene_pool.tile([128, 4, P], f32, tag="yh_sb")
            nc.scalar.copy(yh_sb[:], yh_ps[:])
            st["yh_sb"] = yh_sb

        def mlp1(s, b):
            st = state[s]
            c0 = b * NBLK
            x1 = blk_pool.tile([128, 4, NBLK], f16, tag="x1")
            for m in range(4):
                p1t = p1.tile([128, NBLK], f32, tag="p1")
                nc.tensor.matmul(p1t[:], Wf_sb[:, 0, m * 128:(m + 1) * 128],
                                 st["rel2"][:, 0, c0:c0 + NBLK],
                                 start=True, stop=False)
                nc.tensor.matmul(p1t[:], Wf_sb[:, 1, m * 128:(m + 1) * 128],
                                 st["rel2"][:, 1, c0:c0 + NBLK],
                                 start=False, stop=True)
                nc.vector.tensor_tensor(
                    x1[:, m, :].rearrange("p (i j) -> p i j", i=IB),
                    p1t[:].rearrange("p (i j) -> p i j", i=IB),
                    st["yh_sb"][:, m, :].unsqueeze(1).broadcast_to([128, IB, P]),
                    AluOpType.add)
                nc.scalar.activation(x1[:, m, :], x1[:, m, :], AF.Relu)
            return x1

        def mlp2(s, b, x1):
            st = state[s]
            for mm in range(8):
                p2t = p2.tile([128, 512], f32, tag="p2")
                for k in range(4):
                    nc.tensor.matmul(
                        p2t[:, :NBLK],
                        W2_sb[:, k, mm * 128:(mm + 1) * 128],
                        x1[:, k, :], start=(k == 0), stop=(k == 3))
                nc.vector.tensor_reduce(
                    st["pooled"][:, mm, b * IB:(b + 1) * IB],
                    p2t[:, :NBLK].rearrange("p (i j) -> p i j", i=IB),
                    axis=AX.X, op=AluOpType.max)
            if b == NB - 1:
                for mm in range(8):
                    nc.scalar.activation(
                        st["out_sb"][:, mm, :], st["pooled"][:, mm, :],
                        AF.Relu, bias=b2_sb[:, mm:mm + 1])
                nc.sync.dma_start(d_out[s], st["out_sb"][:])
                state.pop(s)

        # two-deep software pipeline on PE: ... mlp1(i)  mlp2(i-1) ...
        # scene DMAs+prep prefetched a full scene early (GPSIMD prep of a
        # scene takes ~34us); y_h matmuls land mid-scene so the PE never
        # waits on their input DMAs.
        setup_scene0()
        setup_yh(0)
        nc.sync.dma_start(b2_sb[:], d_b2[:])
        for idx, (s, b) in enumerate(blocks):
            if b == 0 and s + 1 < n_scenes:
                setup_dma(s + 1)
            if b == 3 and s + 1 < n_scenes:
                setup_yh(s + 1)
            mlp_q.append((s, b, mlp1(s, b)))
            if len(mlp_q) > 1:
                mlp2(*mlp_q.pop(0))
        mlp2(*mlp_q.pop(0))

    nc.compile()
    return nc


def _host_inputs(h_states, traj, traj_weight, consts, n_scenes=SC):
    """Slice + lay out per-core input maps (all matmul operands f16)."""
    f16 = np.float16
    h_states = np.asarray(h_states, np.float32)
    traj = np.asarray(traj, np.float32)
    traj_weight = np.asarray(traj_weight, np.float32)

    obs = traj[:T].transpose(1, 0, 2).reshape(S, P, 2 * T)   # (S,P,16) g=t*2+c
    h_full = h_states.reshape(S, P, H)

    # obsT[s, p=(r*16+g), j] = obs[s, j, g]   (replica r = 0..7)
    obsT = np.tile(obs.transpose(0, 2, 1), (1, 8, 1))        # (S,128,48)
    combo = np.empty((S, 128, 4 * PP), f16)
    combo[:, :, O_J:O_J + PP] = np.tile(obsT, (1, 1, P))     # [..., i*P+j]=obs[j]
    combo[:, :, O_I:O_I + PP] = np.repeat(obsT, P, axis=2)   # [..., i*P+j]=obs[i]
    # twX[s, p=(r*16+g), col] = tw[s, ct, col], ct = r (tw0) / 8+r (tw1)
    twT = np.ascontiguousarray(
        traj_weight.transpose(0, 2, 3, 1).reshape(S, 16, PP))
    combo[:, :, O_T0:O_T0 + PP] = np.repeat(twT[:, 0:8], 16, axis=1)
    combo[:, :, O_T1:O_T1 + PP] = np.repeat(twT[:, 8:16], 16, axis=1)
    # h_scp[s, k, j] = h[s, j, k] padded to K=128 with the bias row at 64
    h_scp = np.zeros((S, 128, P), f16)
    h_scp[:, :64] = h_full.transpose(0, 2, 1)
    h_scp[:, 64] = 1.0

    c16 = {k: (v.astype(f16) if k in ("Wf_sb", "W1hp", "W2_sb") else v)
           for k, v in consts.items()}

    in_maps = []
    for core in range(NCORES):
        sl = slice(core * n_scenes, (core + 1) * n_scenes)
        m = dict(combo=np.ascontiguousarray(combo[sl]),
                 h_scp=np.ascontiguousarray(h_scp[sl]))
        m.update(c16)
        in_maps.append(m)
    return in_maps


def kernel(h_states, seq_start_end, end_pos, traj, traj_weight,
           mlp_pre_pool_dim_0, W_se, b_se, W1, b1, W2, b2):
    import sys
    if '/opt/trn_rl_repo' not in sys.path:
        sys.path.insert(0, '/opt/trn_rl_repo')
    from concourse.bass_utils import run_bass_kernel_spmd

    consts = _host_constants(W_se, W1, W2, b1, b2)
    in_maps = _host_inputs(h_states, traj, traj_weight, consts)
    nc = build_program(SC)
    res = run_bass_kernel_spmd(nc, in_maps, list(range(NCORES)))
    # device output: [n_scenes, 128 (d%128), 8 (d//128), 48 (i)] f16 per core
    parts = []
    for i in range(NCORES):
        o = np.asarray(res.results[i]["out"], np.float32)
        parts.append(o.transpose(0, 3, 2, 1).reshape(SC * P, D2))
    return np.concatenate(parts, axis=0)


# revision 16
# speedup vs baseline: 1.3873x; 1.0135x over previous
# Trainium2 Bass kernel for nn_Decoder_14568529068506 (gnn_message_passing).
#
# Reference computation (per scene s of 32, P=48 peds):
#   rel[i,j]  = obs[j] - obs[i]                  (P,P,2T)   2T=16
#   emb       = rel @ W_se.T                     (P,P,512)
#   emb      *= tile(traj_weight[s])             (P,P,512)
#   x         = concat([emb, h[j]], -1)          (P,P,576)
#   x1        = relu(x @ W1.T + b1)              (P,P,512)
#   x2        = relu(x1 @ W2.T + b2)             (P,P,1024)
#   out[s,i]  = max_j x2[i,j]                    (P,1024)
#
# Kernel restructuring (validated in fp32 numpy):
#  * traj_weight tiling + spatial embedding + W1 fused on the host:
#      out1[d,row] = sum_{(ct,g)} Wf[d,(ct,g)] * tw[row,ct] * rel[row,g]
#    so MLP1 contracts over 256 "rel2" features; the (P,P,512) embedding
#    never exists.
#  * The h-state part of MLP1 is column-constant within a scene:
#    y_h = W1h @ h + b1 is computed once per scene as a tiny N=48 matmul
#    (bias via a constant-1 input row) and injected per block by one DVE
#    add, so the per-pair MLP1 is two K=128 matmuls per m-tile - the PE
#    stream has a single weight row-group config (no switch bubbles).
#  * All data replication (tw -> 128 partitions, obs -> (r,g)x(i,j)
#    layouts) is pure layout, done on the HOST and shipped as ONE packed
#    DMA per scene; the arithmetic rel = obsJ - obsI, rel2 = tw * rel
#    runs on GPSIMD (SBUF-only engine).
#  * relu/bias commute with max-pool; MLP2 outputs are max-pooled out of
#    PSUM by DVE (one reduce per 128-row m-tile); bias+relu run
#    post-pool on the Act engine into an f16 [128,8,48] tile, one output
#    DMA per scene.
#  * fp16 operands everywhere (1 cycle/row on PE, same speed as bf16,
#    8x the mantissa accuracy of bf16). PSUM accumulation stays fp32.
#  * The host does the final output transpose to (B, 1024) f32 -
#    layout only.
#
# Sharding: scenes are data-parallel across the 8 cores (4 scenes each);
# weights replicated; per-core outputs are concatenated on the host.

import numpy as np

S, P, T, E, H = 32, 48, 8, 64, 64
D1, D2 = 512, 1024
B = S * P
NCORES = 8
SC = S // NCORES          # scenes per core
NB = 6                    # row blocks per scene
NBLK = P * P // NB        # 384 columns (pairs) per block = 8 i-groups x 48 j
IB = NBLK // P            # i-groups per block (8)
PP = P * P                # 2304
# packed per-scene input: chunk-major [NB][obsI | obsJ | tw0 | tw1][NBLK]
# so both the whole-scene DMA and the per-chunk scene-0 DMAs are contiguous


def _host_constants(W_se, W1, W2, b1, b2):
    """Fused weights + lhsT layouts (fp32; cast to f16 in _host_inputs)."""
    W_se = np.asarray(W_se, np.float32)
    W1 = np.asarray(W1, np.float32)
    W2 = np.asarray(W2, np.float32)
    b1 = np.asarray(b1, np.float32)
    b2 = np.asarray(b2, np.float32)

    W1e, W1h = W1[:, :512], W1[:, 512:]
    Wf = np.zeros((D1, 256), np.float32)
    for c in range(2):
        for t in range(T):
            ct = c * 8 + t
            f = t * 64 + np.arange(c, 64, 2)
            Wf[:, ct * 16:(ct + 1) * 16] = W1e[:, f] @ W_se[f, :]

    # lhsT tile layouts: [K(128 part), kTiles, M]
    Wf_sb = np.ascontiguousarray(Wf.T.reshape(2, 128, D1).transpose(1, 0, 2))
    # W1h padded to K=128: row 64 is the constant-1 bias row carrying b1
    W1hp = np.zeros((128, D1), np.float32)
    W1hp[:64] = W1h.T
    W1hp[64] = b1
    W2_sb = np.ascontiguousarray(W2.T.reshape(4, 128, D2).transpose(1, 0, 2))
    b2_sb = np.ascontiguousarray(b2.reshape(8, 128).T)       # (128, 8)
    return dict(Wf_sb=Wf_sb, W1hp=W1hp, W2_sb=W2_sb, b2_sb=b2_sb)


def build_program(n_scenes=SC):
    """Emit the per-core Bass/Tile program. Returns the compiled Bacc."""
    from contextlib import ExitStack
    import concourse.bacc as bacc
    import concourse.tile as tile
    from concourse import mybir
    from concourse.alu_op_type import AluOpType

    f32 = mybir.dt.float32
    f16 = mybir.dt.float16
    AF = mybir.ActivationFunctionType
    AX = mybir.AxisListType

    nc = bacc.Bacc("TRN2", target_bir_lowering=False, debug=False)

    # ---- DRAM parameters -------------------------------------------------
    d_combo = nc.dram_tensor("combo", [n_scenes, 128, NB, 4, NBLK], f16, kind="ExternalInput")
    d_h = nc.dram_tensor("h_scp", [n_scenes, 128, P], f16, kind="ExternalInput")
    d_Wf = nc.dram_tensor("Wf_sb", [128, 2, D1], f16, kind="ExternalInput")
    d_W1hp = nc.dram_tensor("W1hp", [128, D1], f16, kind="ExternalInput")
    d_W2 = nc.dram_tensor("W2_sb", [128, 4, D2], f16, kind="ExternalInput")
    d_b2 = nc.dram_tensor("b2_sb", [128, 8], f32, kind="ExternalInput")
    d_out = nc.dram_tensor("out", [n_scenes, 128, 8, P], f16, kind="ExternalOutput")

    with ExitStack() as ctx:
        tc = ctx.enter_context(tile.TileContext(nc))
        consts = ctx.enter_context(tc.tile_pool(name="consts", bufs=1))
        scene_pool = ctx.enter_context(tc.tile_pool(name="scene", bufs=2))
        blk_pool = ctx.enter_context(tc.tile_pool(name="blk", bufs=3))
        p1 = ctx.enter_context(tc.tile_pool(name="p1", bufs=3, space="PSUM"))
        p2 = ctx.enter_context(tc.tile_pool(name="p2", bufs=4, space="PSUM"))
        pyh = ctx.enter_context(tc.tile_pool(name="pyh", bufs=1, space="PSUM"))

        # weight DMAs ordered for the startup critical path: W1hp (y_h of
        # scene 0) first; Wf (first MLP1) and W2 are interleaved with the
        # scene-0 chunk DMAs below, each just ahead of first use.
        W1hp_sb = consts.tile([128, D1], f16)
        nc.sync.dma_start(W1hp_sb[:], d_W1hp[:])
        Wf_sb = consts.tile([128, 2, D1], f16)
        W2_sb = consts.tile([128, 4, D2], f16)
        b2_sb = consts.tile([128, 8], f32)

        blocks = [(s, b) for s in range(n_scenes) for b in range(NB)]
        state = {}   # per-scene tiles
        mlp_q = []   # software pipeline: deferred MLP2 stage

        def prep_chunk(eng, st, ch):
            combo, rel, rel2 = st["combo"], st["rel"], st["rel2"]
            cs = slice(ch * NBLK, (ch + 1) * NBLK)
            eng.tensor_tensor(rel[:, cs], combo[:, ch, 1, :],
                              combo[:, ch, 0, :], AluOpType.subtract)
            eng.tensor_tensor(rel2[:, 0, cs], rel[:, cs],
                              combo[:, ch, 2, :], AluOpType.mult)
            eng.tensor_tensor(rel2[:, 1, cs], rel[:, cs],
                              combo[:, ch, 3, :], AluOpType.mult)

        def new_scene(s):
            st = state[s] = dict(
                h_scp=scene_pool.tile([128, P], f16, tag="h_scp", name="h_scp"),
                combo=scene_pool.tile([128, NB, 4, NBLK], f16, tag="combo", name="combo"),
                rel=scene_pool.tile([128, PP], f16, tag="rel", name="rel"),
                rel2=scene_pool.tile([128, 2, PP], f16, tag="rel2", name="rel2"),
                pooled=scene_pool.tile([128, 8, P], f32, tag="pooled", name="pooled"),
                out_sb=scene_pool.tile([128, 8, P], f16, tag="out_sb", name="out_sb"))
            nc.sync.dma_start(st["h_scp"][:], d_h[s])
            return st

        def setup_dma(s):
            """DMA the scene's host-packed tiles; GPSIMD builds rel2."""
            st = new_scene(s)
            nc.sync.dma_start(st["combo"][:], d_combo[s])
            for ch in range(NB):
                prep_chunk(nc.gpsimd, st, ch)

        def setup_scene0():
            """Scene 0 arrives in contiguous block-aligned chunks; the
            first chunks' prep runs on DVE (fast, idle at start), the
            rest on GPSIMD. Wf/W2 weight DMAs are slotted where the
            startup critical path wants them."""
            st = new_scene(0)
            for ch in range(NB):
                nc.sync.dma_start(st["combo"][:, ch], d_combo[0][:, ch])
                prep_chunk(nc.vector if ch <= 1 else nc.gpsimd, st, ch)
                if ch == 0:
                    nc.sync.dma_start(Wf_sb[:], d_Wf[:])
                elif ch == 1:
                    nc.sync.dma_start(W2_sb[:], d_W2[:])

        def setup_yh(s, warmup=False):
            """y_h[m] = W1h @ h + b1 as four tiny N=48 matmuls."""
            st = state[s]
            yh_ps = pyh.tile([128, 4, P], f32, tag="yh")
            if warmup:
                # dummy matmuls so the PE p-state ramp (3us of continuous
                # execution -> 2.4GHz) completes during the startup DMAs
                for _ in range(48):
                    nc.tensor.matmul(yh_ps[:, 0, :], W1hp_sb[:, :128],
                                     W1hp_sb[:, :P], start=True, stop=True)
            for m in range(4):
                nc.tensor.matmul(yh_ps[:, m, :],
                                 W1hp_sb[:, m * 128:(m + 1) * 128],
                                 st["h_scp"][:], start=True, stop=True)
            yh_sb = scene_pool.tile([128, 4, P], f32, tag="yh_sb")
            nc.scalar.copy(yh_sb[:], yh_ps[:])
            st["yh_sb"] = yh_sb

        def mlp1(s, b):
            st = state[s]
            c0 = b * NBLK
            x1 = blk_pool.tile([128, 4, NBLK], f16, tag="x1")
            for m in range(4):
                p1t = p1.tile([128, NBLK], f32, tag="p1")
                nc.tensor.matmul(p1t[:], Wf_sb[:, 0, m * 128:(m + 1) * 128],
                                 st["rel2"][:, 0, c0:c0 + NBLK],
                                 start=True, stop=False)
                nc.tensor.matmul(p1t[:], Wf_sb[:, 1, m * 128:(m + 1) * 128],
                                 st["rel2"][:, 1, c0:c0 + NBLK],
                                 start=False, stop=True)
                nc.vector.tensor_tensor(
                    x1[:, m, :].rearrange("p (i j) -> p i j", i=IB),
                    p1t[:].rearrange("p (i j) -> p i j", i=IB),
                    st["yh_sb"][:, m, :].unsqueeze(1).broadcast_to([128, IB, P]),
                    AluOpType.add)
                nc.scalar.activation(x1[:, m, :], x1[:, m, :], AF.Relu)
            return x1

        def mlp2(s, b, x1):
            st = state[s]
            for mm in range(8):
                p2t = p2.tile([128, 512], f32, tag="p2")
                for k in range(4):
                    nc.tensor.matmul(
                        p2t[:, :NBLK],
                        W2_sb[:, k, mm * 128:(mm + 1) * 128],
                        x1[:, k, :], start=(k == 0), stop=(k == 3))
                nc.vector.tensor_reduce(
                    st["pooled"][:, mm, b * IB:(b + 1) * IB],
                    p2t[:, :NBLK].rearrange("p (i j) -> p i j", i=IB),
                    axis=AX.X, op=AluOpType.max)
            if b == NB - 1:
                for mm in range(8):
                    nc.scalar.activation(
                        st["out_sb"][:, mm, :], st["pooled"][:, mm, :],
                        AF.Relu, bias=b2_sb[:, mm:mm + 1])
                nc.sync.dma_start(d_out[s], st["out_sb"][:])
                state.pop(s)

        # two-deep software pipeline on PE: ... mlp1(i)  mlp2(i-1) ...
        # scene DMAs+prep prefetched a full scene early (GPSIMD prep of a
        # scene takes ~34us); y_h matmuls land mid-scene so the PE never
        # waits on their input DMAs.
        setup_scene0()
        setup_yh(0, warmup=True)
        nc.sync.dma_start(b2_sb[:], d_b2[:])
        for idx, (s, b) in enumerate(blocks):
            if b == 0 and s + 1 < n_scenes:
                setup_dma(s + 1)
            if b == 3 and s + 1 < n_scenes:
                setup_yh(s + 1)
            mlp_q.append((s, b, mlp1(s, b)))
            if len(mlp_q) > 1:
                mlp2(*mlp_q.pop(0))
        mlp2(*mlp_q.pop(0))

    nc.compile()
    return nc


def _host_inputs(h_states, traj, traj_weight, consts, n_scenes=SC):
    """Slice + lay out per-core input maps (all matmul operands f16)."""
    f16 = np.float16
    h_states = np.asarray(h_states, np.float32)
    traj = np.asarray(traj, np.float32)
    traj_weight = np.asarray(traj_weight, np.float32)

    obs = traj[:T].transpose(1, 0, 2).reshape(S, P, 2 * T)   # (S,P,16) g=t*2+c
    h_full = h_states.reshape(S, P, H)

    # obsT[s, p=(r*16+g), j] = obs[s, j, g]   (replica r = 0..7)
    obsT = np.tile(obs.transpose(0, 2, 1), (1, 8, 1))        # (S,128,48)
    # twX[s, p=(r*16+g), col] = tw[s, ct, col], ct = r (tw0) / 8+r (tw1)
    twT = np.ascontiguousarray(
        traj_weight.transpose(0, 2, 3, 1).reshape(S, 16, PP))
    combo = np.empty((S, 128, NB, 4, NBLK), f16)
    cv = combo.reshape(S, 128, NB, 4, NBLK)
    cv[:, :, :, 0] = np.repeat(obsT, P, axis=2).reshape(S, 128, NB, NBLK)
    cv[:, :, :, 1] = np.tile(obsT, (1, 1, P)).reshape(S, 128, NB, NBLK)
    cv[:, :, :, 2] = np.repeat(twT[:, 0:8], 16, axis=1).reshape(S, 128, NB, NBLK)
    cv[:, :, :, 3] = np.repeat(twT[:, 8:16], 16, axis=1).reshape(S, 128, NB, NBLK)
    # h_scp[s, k, j] = h[s, j, k] padded to K=128 with the bias row at 64
    h_scp = np.zeros((S, 128, P), f16)
    h_scp[:, :64] = h_full.transpose(0, 2, 1)
    h_scp[:, 64] = 1.0

    c16 = {k: (v.astype(f16) if k in ("Wf_sb", "W1hp", "W2_sb") else v)
           for k, v in consts.items()}

    in_maps = []
    for core in range(NCORES):
        sl = slice(core * n_scenes, (core + 1) * n_scenes)
        m = dict(combo=np.ascontiguousarray(combo[sl]),
                 h_scp=np.ascontiguousarray(h_scp[sl]))
        m.update(c16)
        in_maps.append(m)
    return in_maps


def kernel(h_states, seq_start_end, end_pos, traj, traj_weight,
           mlp_pre_pool_dim_0, W_se, b_se, W1, b1, W2, b2):
    import sys
    if '/opt/trn_rl_repo' not in sys.path:
        sys.path.insert(0, '/opt/trn_rl_repo')
    from concourse.bass_utils import run_bass_kernel_spmd

    consts = _host_constants(W_se, W1, W2, b1, b2)
    in_maps = _host_inputs(h_states, traj, traj_weight, consts)
    nc = build_program(SC)
    res = run_bass_kernel_spmd(nc, in_maps, list(range(NCORES)))
    # device output: [n_scenes, 128 (d%128), 8 (d//128), 48 (i)] f16 per core
    parts = []
    for i in range(NCORES):
        o = np.asarray(res.results[i]["out"], np.float32)
        parts.append(o.transpose(0, 3, 2, 1).reshape(SC * P, D2))
    return np.concatenate(parts, axis=0)
